# revision 13
# baseline (speedup 1.0000x reference)
"""Trainium2 Bass kernel for nn_BasicFlow (sparse window attention flow), v2.

Sharding: pure data-parallel over batch B=8 -> one image pair per NeuronCore.

Device per core:
  - 4x conv3x3 (128->128ch, 96x96) in fp8e4m3 with DoubleRow perf mode:
    weights pre-scaled x16 into e4m3 normal range (they sit in the e4m3
    subnormal range otherwise). 6 DR matmuls per 4-row tile over
    host-pre-shifted input slots [rx_S00, x8_S00, x8_S10, rx_S10]:
    3 feature-residual pairs (x8*A + 16(x-x8)*A/16) for the row-0 taps
    and 3 pure-x8 vertical tap pairs (1,dx)+(2,dx). Bias is added in
    f32 on the psum drain copies (DVE tensor_scalar_add / Act
    activation), not on the PE.
  - conv outputs written as bf16 "extended" images ext[128,100,100]
    (4 wrap rows/cols) so every shifted window is a natural 3D slice.
  - correlation: per shift-variant window-major q tiles (stationary must be
    a single contiguous free dim); moving k operand sliced directly from
    k-ext. 8 variants x 144 windows, 64x64 each, bf16, two windows per
    psum bank via tile_position column groups.
  - corr staged to fp16 (values ~256x raw; host divides) and DMA'd out.
Host tail: bias/mask + softmax flow pipeline + splice + bilinear upsample.
"""

import os

# recover wedged NeuronCores at NRT init (observed transient
# NRT_EXEC_UNIT_UNRECOVERABLE; reset-on-load clears it)
os.environ.setdefault("NEURON_RT_RESET_CORES", "1")

import numpy as np
import ml_dtypes

import concourse.bass as bass
import concourse.bacc as bacc
import concourse.tile as tile
import concourse.mybir as mybir
from concourse import bass_utils

F32 = mybir.dt.float32
BF16 = mybir.dt.bfloat16
FP16 = mybir.dt.float16
FP8 = mybir.dt.float8e4
DR = mybir.MatmulPerfMode.DoubleRow

B = 8
DIM = 128
H = W = 96
P = 8
UP = 4
SCALE = DIM ** -0.5
S1 = S2 = H // P          # 12 windows per axis
NW = S1 * S2              # 144 windows
NV = 8                    # 4 shift variants x 2 directions
RT = 4                    # conv output rows per psum tile
NRT = H // RT
WSCALE = 16.0             # fp8 weight pre-scale; corr comes out x WSCALE^2
EXT = 100                 # extended image height/width (4 wrap rows/cols)

_COMPILED = None


# --------------------------------------------------------------------------
# Device kernel
# --------------------------------------------------------------------------

def _build_device():
    nc = bacc.Bacc("TRN2", target_bir_lowering=False, debug=False, num_devices=8)

    # slots per map: [x8_S00, rx16_S00, x8_S10, rx16_S10], heights 97, w 98
    s0_d = nc.dram_tensor("s0", [DIM, 4, 97, 98], FP8, kind="ExternalInput")
    s2_d = nc.dram_tensor("s2", [DIM, 4, 97, 98], FP8, kind="ExternalInput")
    # weights: 6 tap matmuls x 2 k-tiles x 128 out-ch; biases (x WSCALE) in f32
    wq_d = nc.dram_tensor("wq", [DIM, 6, 2, DIM], FP8, kind="ExternalInput")
    wk_d = nc.dram_tensor("wk", [DIM, 6, 2, DIM], FP8, kind="ExternalInput")
    bq_d = nc.dram_tensor("bq", [DIM, 1], F32, kind="ExternalInput")
    bk_d = nc.dram_tensor("bk", [DIM, 1], F32, kind="ExternalInput")
    # raw correlation volumes, window pairs packed across 128 partitions:
    # [variant*2+dir, par*64+q_pixel, window_pair, k_pixel], window = 2*pair+par
    corr_d = nc.dram_tensor("corr", [NV, 2 * P * P, NW // 2, P * P], FP16,
                            kind="ExternalOutput")

    with tile.TileContext(nc) as tc:
        with (
            tc.tile_pool(name="const", bufs=1) as constp,
            tc.tile_pool(name="slots", bufs=2) as slotp,
            tc.tile_pool(name="ext", bufs=1) as extp,
            tc.tile_pool(name="wm0", bufs=1) as wm0p,
            tc.tile_pool(name="stage", bufs=3) as stagep,
            tc.tile_pool(name="psum", bufs=8, space="PSUM") as psump,
        ):
            wq_sb = constp.tile([DIM, 6, 2, DIM], FP8, tag="wq")
            wk_sb = constp.tile([DIM, 6, 2, DIM], FP8, tag="wk")
            bq_sb = constp.tile([DIM, 1], F32, tag="bq")
            bk_sb = constp.tile([DIM, 1], F32, tag="bk")
            # chunked slot loads so convs can start early; interleave the
            # issue order so the first conv's deps (wq + slot rows 0..8)
            # land before lower-priority constants
            def load_slots(dst, src_d, bounds):
                for r0, r1 in zip(bounds, bounds[1:]):
                    nc.sync.dma_start(dst[:, :, r0:r1, :], src_d[:, :, r0:r1, :])

            slots0 = slotp.tile([DIM, 4, 97, 98], FP8, tag="slots")
            slots2 = slotp.tile([DIM, 4, 97, 98], FP8, tag="slots")
            nc.sync.dma_start(wq_sb[:], wq_d[:])
            load_slots(slots0, s0_d, [0, 12, 28])
            nc.sync.dma_start(bq_sb[:], bq_d[:])
            load_slots(slots0, s0_d, [28, 52, 76, 97])
            nc.sync.dma_start(wk_sb[:], wk_d[:])
            nc.sync.dma_start(bk_sb[:], bk_d[:])
            load_slots(slots2, s2_d, [0, 28, 52, 76, 97])

            q0e = extp.tile([DIM, EXT, EXT], BF16, tag="q0e")
            k0e = extp.tile([DIM, EXT, EXT], BF16, tag="k0e")
            q2e = extp.tile([DIM, EXT, EXT], BF16, tag="q2e")
            k2e = extp.tile([DIM, EXT, EXT], BF16, tag="k2e")
            # variant-0 window-major tile for q0, filled straight from psum
            wm0_q0 = wm0p.tile([DIM, NW, P * P], BF16, tag="wm0")

            psi = [0]

            def bias_copy(dst, src, b_sb):
                # psum->sbuf with per-partition bias add (GPSIMD lacks PSUM)
                if psi[0] % 2 == 0:
                    nc.vector.tensor_scalar_add(dst, src, b_sb[:])
                else:
                    nc.scalar.activation(dst, src,
                                         mybir.ActivationFunctionType.Identity,
                                         bias=b_sb[:])
                psi[0] += 1

            def rr_psum_copy(dst, src):
                if psi[0] % 2 == 0:
                    nc.vector.tensor_copy(dst, src)
                else:
                    nc.scalar.copy(dst, src)
                psi[0] += 1

            cpi = [0]
            cpeng = [nc.vector.tensor_copy, nc.scalar.copy,
                     nc.gpsimd.tensor_copy]

            def rr_copy(dst, src):
                # SBUF->SBUF: any engine
                cpeng[cpi[0] % 3](dst, src)
                cpi[0] += 1

            def conv(dst, slots, w_sb, b_sb, wm0=None):
                """3x3 conv (fp8 DoubleRow) + bias on the psum drain copy."""
                wmv = None
                if wm0 is not None:
                    wmv = wm0[:].rearrange(
                        "p (wy wx) (ly lx) -> p wy ly wx lx", wx=S2, lx=P)
                # 6 DR matmuls: 3 feature-residual pairs (rx,x8 slots) for
                # row-0 taps; 3 pure-x8 vertical tap pairs (1,dx)+(2,dx)
                specs = ((0, 0, 0), (0, 0, 1), (0, 0, 2),
                         (1, 1, 0), (1, 1, 1), (1, 1, 2))
                for rt in range(NRT):
                    y = rt * RT
                    ps = psump.tile([DIM, RT, W], F32, tag="ps")
                    for m, (sl, yo, dx) in enumerate(specs):
                        nc.tensor.matmul(
                            ps[:], w_sb[:, m, :, :],
                            slots[:, sl:sl + 2, y + yo:y + yo + RT, dx:dx + W],
                            start=(m == 0), stop=(m == 5), perf_mode=DR)
                    bias_copy(dst[:, y:y + RT, 0:W], ps[:], b_sb)
                    if wmv is not None:
                        bias_copy(wmv[:, y // P, y % P:y % P + RT], ps[:], b_sb)

            def borders(e):
                # wrap rows then wrap cols (incl corner)
                rr_copy(e[:, H:EXT, 0:W], e[:, 0:EXT - H, 0:W])
                rr_copy(e[:, 0:EXT, W:EXT], e[:, 0:EXT, 0:EXT - W])

            conv(q0e, slots0, wq_sb, bq_sb, wm0=wm0_q0)
            borders(q0e)
            conv(k0e, slots0, wk_sb, bk_sb)   # f0 slots die after this
            borders(k0e)
            conv(q2e, slots2, wq_sb, bq_sb)
            borders(q2e)
            conv(k2e, slots2, wk_sb, bk_sb)   # f2 slots die after this
            borders(k2e)

            # window-major stationary tiles (5D copies: DVE/GPSIMD only;
            # GPSIMD is ~2.75x slower, give it the smaller share)
            def wm_copy(dst, src, ry, rx):
                s = src[:, ry:ry + H, rx:rx + W].rearrange(
                    "p (wy ly) (wx lx) -> p wy wx ly lx", ly=P, lx=P)
                d = dst[:].rearrange(
                    "p (wy wx) (ly lx) -> p wy wx ly lx", wx=S2, lx=P)
                nc.vector.tensor_copy(d[:, 0:9], s[:, 0:9])
                nc.gpsimd.tensor_copy(d[:, 9:S1], s[:, 9:S1])

            SH = ((0, 0), (0, 4), (4, 0), (4, 4))
            WG = 16
            NG = NW // WG                      # 9 psum groups per variant

            def corr_variant(vd, qwm, ke, ry, rx):
                sb = stagep.tile([2 * P * P, NW // 2, P * P], FP16,
                                 tag="corrsb")
                for wg in range(NG):
                    ps = psump.tile([2 * P * P, WG // 2, P * P], F32, tag="ps")
                    for wi in range(WG // 2):
                        w = wg * WG + 2 * wi
                        for par in range(2):
                            i, j = divmod(w + par, S2)
                            nc.tensor.matmul(
                                ps[par * 64:(par + 1) * 64, wi, :],
                                qwm[:, w + par, :],
                                ke[:, ry + 8 * i:ry + 8 * i + 8,
                                   rx + 8 * j:rx + 8 * j + 8],
                                start=True, stop=True,
                                tile_position=(0, 64 * par))
                    rr_psum_copy(sb[:, wg * (WG // 2):(wg + 1) * (WG // 2), :],
                                 ps[:])
                    # drain eagerly in 4 chunks; the last one is small so the
                    # post-PE DMA tail stays short
                    if wg == 2:
                        nc.sync.dma_start(corr_d[vd, :, 0:24, :],
                                          sb[:, 0:24, :])
                    elif wg == 5:
                        nc.sync.dma_start(corr_d[vd, :, 24:48, :],
                                          sb[:, 24:48, :])
                    elif wg == 7:
                        nc.sync.dma_start(corr_d[vd, :, 48:64, :],
                                          sb[:, 48:64, :])
                nc.sync.dma_start(corr_d[vd, :, 64:NW // 2, :],
                                  sb[:, 64:NW // 2, :])

            wms = {(0, 0): wm0_q0}
            for d, qe in ((0, q0e), (1, q2e)):
                for v, (ry, rx) in enumerate(SH):
                    if (d, v) != (0, 0):
                        t = slotp.tile([DIM, NW, P * P], BF16, tag="slots")
                        wm_copy(t, qe, ry, rx)
                        wms[(d, v)] = t
                    # emit corr one variant behind to keep the wm pipeline
                    # ahead of the PE
                    prev = (d, v - 1) if v else (d - 1, len(SH) - 1)
                    if prev in wms:
                        pd, pv = prev
                        pry, prx = SH[pv]
                        corr_variant(pv * 2 + pd, wms[prev],
                                     k2e if pd == 0 else k0e, pry, prx)
                        del wms[prev]
            corr_variant(3 * 2 + 1, wms[(1, 3)], k0e, *SH[3])

    nc.compile()
    return nc


# --------------------------------------------------------------------------
# Host-side input prep + device run
# --------------------------------------------------------------------------

def _prep_slots(img):
    """img: (128, 96, 96) f32 -> (128, 4, 97, 98) fp8 slot tensor."""
    f8 = ml_dtypes.float8_e4m3
    pad = np.zeros((DIM, H + 2, W + 3), np.float32)
    pad[:, 1:H + 1, 1:W + 1] = img
    x8 = pad.astype(f8)
    rx = ((pad - x8.astype(np.float32)) * WSCALE).astype(f8)
    out = np.empty((DIM, 4, 97, 98), f8)
    out[:, 0] = rx[:, 0:97, 0:98]
    out[:, 1] = x8[:, 0:97, 0:98]
    out[:, 2] = x8[:, 1:98, 0:98]
    out[:, 3] = rx[:, 1:98, 0:98]
    return out


def _prep_weights(wgt):
    """wgt: (co, ci, 3, 3) -> (128, 6, 2, 128) fp8.

    m0-2: resid pairs (rx*A/16, x8*A) for taps (0,dx);
    m3-5: pure vertical pairs (A(1,dx), A(2,dx)).
    """
    f8 = ml_dtypes.float8_e4m3

    def a16(dy, dx):
        return (wgt[:, :, dy, dx].T * WSCALE).astype(f8).astype(np.float32)

    out = np.zeros((DIM, 6, 2, DIM), np.float32)
    for m in range(3):
        A = a16(0, m)
        out[:, m, 0, :] = A / WSCALE
        out[:, m, 1, :] = A
    for m in range(3):
        out[:, 3 + m, 0, :] = a16(1, m)
        out[:, 3 + m, 1, :] = a16(2, m)
    return out.astype(f8)


def _run_device(feat0, feat2, wq, bq, wk, bk):
    global _COMPILED
    if _COMPILED is None:
        _COMPILED = _build_device()
    nc = _COMPILED

    wqp = _prep_weights(wq.astype(np.float32))
    wkp = _prep_weights(wk.astype(np.float32))
    bqp = np.ascontiguousarray((bq.astype(np.float32) * WSCALE).reshape(DIM, 1))
    bkp = np.ascontiguousarray((bk.astype(np.float32) * WSCALE).reshape(DIM, 1))

    in_maps = []
    for b in range(B):
        in_maps.append({
            "s0": _prep_slots(feat0[b].astype(np.float32)),
            "s2": _prep_slots(feat2[b].astype(np.float32)),
            "wq": wqp, "wk": wkp, "bq": bqp, "bk": bkp,
        })
    trace = bool(int(os.environ.get("BASSFLOW_TRACE", "0")))
    res = bass_utils.run_bass_kernel_spmd(nc, in_maps, core_ids=list(range(B)),
                                          trace=trace)
    if trace:
        print(f"HW exec time: {res.exec_time_ns} ns "
              f"(mean {res.mean_exec_time_ns})")
        if res.instructions_and_trace:
            print("trace path:", res.instructions_and_trace[1])
    corr = np.stack([res.results[b]["corr"] for b in range(B)])
    # [B, NV, par*64+q, pair, k] -> [B, NV, win=2*pair+par, q, k]
    corr = corr.astype(np.float32) / (WSCALE * WSCALE)
    corr = corr.reshape(B, NV, 2, P * P, NW // 2, P * P)
    corr = corr.transpose(0, 1, 4, 2, 3, 5).reshape(B, NV, NW, P * P, P * P)
    return corr


# --------------------------------------------------------------------------
# Host tail: bias/mask + softmax flow pipeline + splice + bilinear upsample
# (numpy port of the reference; ~1% of total FLOPs)
# --------------------------------------------------------------------------

def _bias_index():
    coords = np.stack(np.meshgrid(np.arange(P), np.arange(P),
                                  indexing='ij')).reshape(2, -1)
    rel = (coords[:, :, None] - coords[:, None, :]).transpose(1, 2, 0).copy()
    rel[..., 0] += P - 1
    rel[..., 1] += P - 1
    rel[..., 0] *= 2 * P - 1
    return rel.sum(-1).reshape(-1)


def _pos():
    r = np.arange(P, dtype=np.float32)
    yy, xx = np.meshgrid(r, r, indexing='ij')
    return np.stack([xx, yy])[None].reshape(1, 2, P * P)


def _make_mask(Hp, Wp, sh, sw):
    m = np.zeros((Hp, Wp))
    hs = ((slice(0, -sh * 2), slice(-sh * 2, -sh), slice(-sh, None))
          if sh else (slice(None),))
    ws = ((slice(0, -sw * 2), slice(-sw * 2, -sw), slice(-sw, None))
          if sw else (slice(None),))
    cnt = 0
    for a in hs:
        for b in ws:
            m[a, b] = cnt
            cnt += 1
    win = m.reshape(Hp // P, P, Wp // P, P).transpose(0, 2, 1, 3).reshape(-1, P * P)
    d = win[:, None, :] - win[:, :, None]
    return np.where(d != 0, -10000.0, 0.0).astype(np.float32)


def _softmax(x, axis):
    m = np.max(x, axis=axis, keepdims=True)
    e = np.exp(x - m)
    return e / np.sum(e, axis=axis, keepdims=True)


_MID_IDX = None


def _mid_gather():
    global _MID_IDX
    if _MID_IDX is None:
        j, k, h2, w2 = np.meshgrid(np.arange(9), np.arange(9), np.arange(P),
                                   np.arange(P), indexing='ij')
        qy = j + 3 - h2
        qx = k + 3 - w2
        valid = (qy >= 0) & (qy < P) & (qx >= 0) & (qx < P)
        qidx = np.clip(qy, 0, P - 1) * P + np.clip(qx, 0, P - 1)
        kidx = h2 * P + w2
        _MID_IDX = (qidx.reshape(81, 64), kidx.reshape(81, 64),
                    valid.reshape(81, 64))
    return _MID_IDX


def _flow_mid(corr, pos):
    bw = corr.shape[0]
    qidx, kidx, valid = _mid_gather()
    c = corr[:, qidx, kidx] * valid[None]          # (bw, 81, 64)
    n = P + 1
    r = np.arange(0.0, P - 0.5, 0.5)
    yy, xx = np.meshgrid(r, r, indexing='ij')
    CH = P // 2 - 1
    base = np.stack([xx, yy])[None][:, :, CH:2 * P - 1 - CH, CH:2 * P - 1 - CH]
    base = base.reshape(1, 2, n * n).astype(np.float32)
    flow = pos[:, :, None, :] - base[:, :, :, None]          # (1,2,81,64)
    smax = _softmax(c, axis=2)
    fl = np.einsum('bmk,cmk->bcm', smax, flow[0]).reshape(bw, 2, n, n)
    cr = np.sum(c * smax, axis=2).reshape(bw, 1, n, n)
    corr4 = np.concatenate([cr[:, :, :-1, :-1], cr[:, :, :-1, 1:],
                            cr[:, :, 1:, :-1], cr[:, :, 1:, 1:]], axis=1)
    flow4 = np.concatenate([fl[:, :, :-1, :-1], fl[:, :, :-1, 1:],
                            fl[:, :, 1:, :-1], fl[:, :, 1:, 1:]], axis=1)
    corr4 = corr4.transpose(0, 2, 3, 1).reshape(bw, P * P, 4)
    flow4 = flow4.reshape(bw, 4, 2, P, P).transpose(0, 2, 3, 4, 1)
    flow4 = flow4.reshape(bw, 2, P * P, 4) * 2
    smax2 = _softmax(corr4, axis=2)
    out = np.sum(flow4 * smax2[:, None], axis=3)
    return out.reshape(bw, 2, P, P).astype(np.float32)


def _flow_bsd(corr, pos):
    cut = P // 4
    bw = corr.shape[0]
    c = corr.reshape(bw, P, P, P * P)[:, cut:P - cut, cut:P - cut, :]
    L = (P - 2 * cut) ** 2
    c = c.reshape(bw, L, P * P)
    base = _pos().reshape(1, 2, P, P)[:, :, cut:P - cut, cut:P - cut]
    base = base.reshape(1, 2, L)
    flow = pos[:, :, None, :] - base[:, :, :, None]
    smax = _softmax(c, axis=2)
    out = np.einsum('blk,clk->bcl', smax, flow[0])
    return out.reshape(bw, 2, P - 2 * cut, P - 2 * cut).astype(np.float32)


def _splice(f00, f01, f10, f11, factor, Ho, Wo):
    f = np.concatenate([np.concatenate([f00, f01], axis=3),
                        np.concatenate([f10, f11], axis=3)], axis=2)
    bs, kk, hh, ww = f.shape
    b = bs // (S1 * S2)
    f = f.reshape(b, S1, S2, kk, hh, ww).transpose(0, 3, 1, 4, 2, 5)
    f = f.reshape(b, kk, S1 * hh, S2 * ww)
    sft = (P // 4) * factor
    f = np.roll(f, (sft, sft), axis=(2, 3))
    return f[:, :, :Ho * factor, :Wo * factor]


def _resize_mat(in_size, out_size):
    scale = out_size / in_size
    sample = (np.arange(out_size) + 0.5) / scale - 0.5
    x = np.abs(sample[None, :] - np.arange(in_size)[:, None])
    w = np.maximum(0.0, 1.0 - x)
    tot = w.sum(0, keepdims=True)
    return (w / np.where(tot == 0, 1.0, tot)).astype(np.float32)


def _up(x, f):
    b, c, h, w = x.shape
    My = _resize_mat(h, h * f)
    Mx = _resize_mat(w, w * f)
    y = np.einsum('bchw,hH->bcHw', x, My)
    y = np.einsum('bcHw,wW->bcHW', y, Mx)
    return (y * f).astype(np.float32)


def _host_flow(corr_raw, bias_table):
    """corr_raw: (B, NV, NW, 64, 64) raw q.k^T dot products."""
    bias = bias_table.astype(np.float32)[_bias_index()].reshape(
        P * P, P * P, 1).transpose(2, 0, 1)          # (1,64,64)
    pos = _pos()
    masks = {}
    for v, (sh, sw) in enumerate(((0, 0), (0, 4), (4, 0), (4, 4))):
        masks[v] = _make_mask(H, W, sh, sw) if (sh or sw) else None

    f1 = {}
    f0 = {}
    for v in range(4):
        for d in range(2):
            c = corr_raw[:, v * 2 + d].reshape(B * NW, 64, 64) * SCALE + bias
            if masks[v] is not None:
                c = (c.reshape(B, NW, 64, 64) + masks[v][None]).reshape(
                    B * NW, 64, 64)
            f1[(v, d)] = _flow_mid(c, pos)
            f0[(v, d)] = _flow_bsd(c, pos)

    flow12 = _splice(f1[(0, 0)], f1[(1, 0)], f1[(2, 0)], f1[(3, 0)], 2, H, W)
    flow02 = _splice(f0[(0, 0)], f0[(1, 0)], f0[(2, 0)], f0[(3, 0)], 1, H, W)
    flow10 = _splice(f1[(0, 1)], f1[(1, 1)], f1[(2, 1)], f1[(3, 1)], 2, H, W)
    flow20 = _splice(f0[(0, 1)], f0[(1, 1)], f0[(2, 1)], f0[(3, 1)], 1, H, W)
    fh, ff = UP // 2, UP
    return (_up(flow10, fh), _up(flow12, fh), _up(flow02, ff), _up(flow20, ff))


def kernel(feat0, feat2, wq, bq, wk, bk, bias_table):
    corr_raw = _run_device(np.asarray(feat0), np.asarray(feat2),
                           np.asarray(wq), np.asarray(bq),
                           np.asarray(wk), np.asarray(bk))
    return _host_flow(corr_raw, np.asarray(bias_table))
